# revision 1
# baseline (speedup 1.0000x reference)
"""GCN (2x GCNConv + linear head) on 8 TRN2 NeuronCores.

Strategy (graph-parallel by target node):
- Nodes are sharded across 8 cores (6250 real + padding = 6400 rows/core,
  table numbering: table_row = core*6400 + local).
- Layer tables H = dis * (x @ W) live in DRAM, rows are 512B (128 f32) so
  dma_gather fetches one edge-message per descriptor at line rate.
- Edges are grouped per (core, 256-target window, lo/hi source range) --
  the lo/hi split works around dma_gather's int16 index range.  Each
  128-edge block becomes one fp32r matmul: PSUM[feat, tgt] +=
  gathered[edge, feat].T @ onehot[edge, tgt], where
  onehot[e, t] = (t == col_in_window[e]) * rsqrt(deg[col[e]]) is built by a
  single DVE tensor_scalar op.  Self-loops are ordinary edges.
- Normalization: table rows are pre-scaled by rsqrt(deg[src]); the target
  factor rides inside the one-hot.  deg is an integer histogram of the
  (index-only) edge list, computed host-side; all float math is on-device.
- Layer 1 table is built redundantly on every core (cheaper than a second
  collective); layer 2 table is AllGathered from per-core shards.
"""

import numpy as np

N_REAL = 50000
E_REAL = 800000
D = 128
NCORES = 8
NO_AG = False
ALL_F32 = False
PHASES = {"A", "G1", "S", "G2", "H"}

_CFG_FULL = dict(n=N_REAL, nsh=6250, win=256, split=32768)


def _derive(cfg):
    nsh = cfg["nsh"]
    nloc = ((nsh + 127) // 128) * 128
    win = cfg["win"]
    nloc = ((nloc + win - 1) // win) * win
    npad = NCORES * nloc
    return nloc, npad, nloc // win, npad // 128


def prep(edge_index, cfg=_CFG_FULL):
    """Host-side (integer-only) graph preprocessing -> per-core arrays."""
    n, nsh, win, split = cfg["n"], cfg["nsh"], cfg["win"], cfg["split"]
    nloc, npad, nwin, _ = _derive(cfg)
    row = np.asarray(edge_index[0]).astype(np.int64)
    col = np.asarray(edge_index[1]).astype(np.int64)

    deg = np.bincount(col, minlength=n).astype(np.float32) + 1.0  # + self loop
    deg_t = np.ones(npad, np.float32)
    rr = np.arange(n, dtype=np.int64)
    t_of_r = (rr // nsh) * nloc + (rr % nsh)
    deg_t[t_of_r] = deg

    trow = (row // nsh) * nloc + (row % nsh)
    tcol = (col // nsh) * nloc + (col % nsh)
    core_of = col // nsh

    # per (core, window, class) group sizes -> global NBL/NBH
    per_core = []
    for c in range(NCORES):
        m = core_of == c
        er = trow[m]
        ecl = tcol[m] - c * nloc
        sl = c * nloc + np.arange(nsh, dtype=np.int64)  # self loops
        er = np.concatenate([er, sl])
        ecl = np.concatenate([ecl, np.arange(nsh, dtype=np.int64)])
        w = ecl // win
        is_hi = (er >= split).astype(np.int64)
        key = w * 2 + is_hi
        order = np.argsort(key, kind="stable")
        er, ecl, key = er[order], ecl[order], key[order]
        bounds = np.searchsorted(key, np.arange(2 * nwin + 1))
        per_core.append((er, ecl, bounds))

    nb = np.zeros((NCORES, nwin, 2), np.int64)
    for c in range(NCORES):
        _, _, bounds = per_core[c]
        for w in range(nwin):
            nb[c, w, 0] = bounds[2 * w + 1] - bounds[2 * w]
            nb[c, w, 1] = bounds[2 * w + 2] - bounds[2 * w + 1]
    nbl = int((nb[:, :, 0].max() + 127) // 128)
    nbh = int(max(1, (nb[:, :, 1].max() + 127) // 128))

    cores = []
    for c in range(NCORES):
        er, ecl, bounds = per_core[c]
        arrs = {}
        for cls, nbx in ((0, nbl), (1, nbh)):
            cap = nbx * 128
            src = np.zeros((nwin, cap), np.int64)
            cw = np.full((nwin, cap), -1.0, np.float32)
            dcol = np.ones((nwin, cap), np.float32)
            for w in range(nwin):
                a, b = bounds[2 * w + cls], bounds[2 * w + cls + 1]
                k = b - a
                s = er[a:b] - (split if cls else 0)
                src[w, :k] = s
                cw[w, :k] = (ecl[a:b] % win).astype(np.float32)
                dcol[w, :k] = deg_t[c * nloc + ecl[a:b]]
            # gather idx layout: edge e -> part e%16 (replicated x8), col e//16
            g16 = src.reshape(nwin, cap // 16, 16).transpose(0, 2, 1)  # [w,16,cap/16]
            gidx = np.tile(g16, (1, 8, 1)).transpose(1, 0, 2).reshape(128, nwin * cap // 16)
            # colw/dcol layout: edge e of block b -> part e%128, col w*nbx+b
            cwt = cw.reshape(nwin * nbx, 128).T.copy()
            dct = dcol.reshape(nwin * nbx, 128).T.copy()
            sfx = "lo" if cls == 0 else "hi"
            arrs[f"gidx_{sfx}"] = gidx.astype(np.int16)
            arrs[f"colw_{sfx}"] = cwt
            arrs[f"dcol_{sfx}"] = dct
        # deg of this core's own table rows, [128, nloc/128] tiled
        arrs["degloc"] = deg_t[c * nloc:(c + 1) * nloc].reshape(-1, 128).T.copy()
        cores.append(arrs)

    degt_t = deg_t.reshape(-1, 128).T.copy()  # [128, npad/128]
    return cores, degt_t, nbl, nbh, t_of_r


def build_nc(nbl, nbh, cfg=_CFG_FULL):
    import concourse.bacc as bacc
    import concourse.tile as tile
    import concourse.mybir as mybir
    from concourse.alu_op_type import AluOpType

    nloc, npad, nwin, ntile = _derive(cfg)
    split = cfg["split"]
    f32 = mybir.dt.float32
    f32r = mybir.dt.float32 if ALL_F32 else mybir.dt.float32r
    AF = mybir.ActivationFunctionType
    nsh_t = nloc // 128  # local tiles
    XCH = 16  # stage-A xT chunk, in 128-col tiles

    nc = bacc.Bacc("TRN2", target_bir_lowering=False, debug=False,
                   num_devices=NCORES)
    inp = {}

    def I(name, shape, dt=f32):
        inp[name] = nc.dram_tensor(name, list(shape), dt, kind="ExternalInput").ap()
        return inp[name]

    xT = I("xT", [128, npad])
    W1 = I("W1", [128, 128]); W2 = I("W2", [128, 128]); Wh = I("Wh", [128, 3])
    b1 = I("b1", [128, 1]); b2 = I("b2", [128, 1]); bh = I("bh", [128, 3])
    degt = I("degt", [128, ntile]); degloc = I("degloc", [128, nsh_t])
    iota = I("iota", [128, cfg["win"]])
    g_lo = I("gidx_lo", [128, nwin * nbl * 8], mybir.dt.int16)
    g_hi = I("gidx_hi", [128, nwin * nbh * 8], mybir.dt.int16)
    c_lo = I("colw_lo", [128, nwin * nbl]); c_hi = I("colw_hi", [128, nwin * nbh])
    d_lo = I("dcol_lo", [128, nwin * nbl]); d_hi = I("dcol_hi", [128, nwin * nbh])
    out = nc.dram_tensor("out", [128, nsh_t * 3], f32, kind="ExternalOutput").ap()

    T1 = nc.dram_tensor("T1", [npad, 128], f32r, kind="Internal").ap()
    bounce = nc.dram_tensor("bounce", [nloc, 128], f32r, kind="Internal").ap()
    T2 = nc.dram_tensor("T2", [npad, 128], f32r, kind="Internal",
                        addr_space=("Local" if NO_AG else "Shared")).ap()

    with tile.TileContext(nc) as tc:
        with (
            tc.tile_pool(name="const", bufs=1) as pc,
            tc.tile_pool(name="xch", bufs=2) as pxch,
            tc.tile_pool(name="ha", bufs=3) as pha,
            tc.tile_pool(name="glo", bufs=2) as pglo,
            tc.tile_pool(name="ghi", bufs=2) as pghi,
            tc.tile_pool(name="oh", bufs=6) as poh,
            tc.tile_pool(name="act", bufs=1) as pact,
            tc.tile_pool(name="psA", bufs=2, space="PSUM") as psA,
            tc.tile_pool(name="psW", bufs=2, space="PSUM") as psW,
            tc.tile_pool(name="psH", bufs=2, space="PSUM") as psH,
        ):
            def load(ap, shape, tag, dt=f32):
                t = pc.tile(shape, dt, tag=tag)
                nc.sync.dma_start(t[:], ap[:])
                return t

            iota_sb = load(iota, [128, cfg["win"]], "iota")
            W1_sb = load(W1, [128, 128], "W1"); W2_sb = load(W2, [128, 128], "W2")
            Wh_sb = load(Wh, [128, 3], "Wh")
            b1_sb = load(b1, [128, 1], "b1"); b2_sb = load(b2, [128, 1], "b2")
            bh_sb = load(bh, [128, 3], "bh")
            glo_sb = load(g_lo, [128, nwin * nbl * 8], "glosb", mybir.dt.int16)
            ghi_sb = load(g_hi, [128, nwin * nbh * 8], "ghisb", mybir.dt.int16)
            clo_sb = load(c_lo, [128, nwin * nbl], "closb")
            chi_sb = load(c_hi, [128, nwin * nbh], "chisb")

            def rsqrt_of(ap, cols, tag):
                dsb = load(ap, [128, cols], tag + "_d")
                rec = pc.tile([128, cols], f32, tag=tag + "_r")
                nc.vector.reciprocal(rec[:], dsb[:])
                o = pc.tile([128, cols], f32, tag=tag + "_o")
                nc.scalar.activation(o[:], rec[:], AF.Sqrt)
                return o

            dis_sb = rsqrt_of(degt, ntile, "dis")
            disloc_sb = rsqrt_of(degloc, nsh_t, "disl")
            slo_sb = rsqrt_of(d_lo, nwin * nbl, "slo")
            shi_sb = rsqrt_of(d_hi, nwin * nbh, "shi")

            # persistent activations (feature-major)
            x2T = pact.tile([128, nloc], f32, tag="x2T")
            x3T = pact.tile([128, nloc], f32, tag="x3T")
            out_sb = pact.tile([128, nsh_t * 3], f32, tag="osb")
            nc.vector.memset(x2T[:], 0.0)
            nc.vector.memset(x3T[:], 0.0)

            # ---- stage A: full layer-1 table on every core ----
            for t in range(ntile if "A" in PHASES else 0):
                if t % XCH == 0:
                    xc = pxch.tile([128, XCH * 128], f32, tag="xch")
                    hi = min(npad, (t + XCH) * 128)
                    nc.sync.dma_start(xc[:, : hi - t * 128], xT[:, t * 128: hi])
                ps = psA.tile([128, 128], f32, tag="psA")
                nc.tensor.matmul(ps[:], xc[:, (t % XCH) * 128:(t % XCH + 1) * 128],
                                 W1_sb[:], start=True, stop=True)
                h = pha.tile([128, 128], f32r, tag="ha")
                nc.vector.tensor_scalar(h[:], ps[:], dis_sb[:, t:t + 1], None,
                                        AluOpType.mult)
                nc.sync.dma_start(T1[t * 128:(t + 1) * 128, :], h[:])

            # ---- one GCN aggregation layer ----
            GCH = 8  # max 1024 descriptors per dma_gather call

            def agg_layer(T, xTnext, bias_sb):
                for w in range(nwin):
                    parts = []
                    for cls, pl, nbx, lim, gsb, csb, ssb in (
                            (0, pglo, nbl, (0, split), glo_sb, clo_sb, slo_sb),
                            (1, pghi, nbh, (split, npad), ghi_sb, chi_sb, shi_sb)):
                        for s0 in range(0, nbx, GCH):
                            cs = min(GCH, nbx - s0)
                            gt = pl.tile([128, cs, 128], f32r, tag=f"g{cls}_{s0}")
                            o0 = (w * nbx + s0) * 8
                            nc.gpsimd.dma_gather(
                                gt[:], T[lim[0]:lim[1], :], gsb[:, o0:o0 + cs * 8],
                                num_idxs=cs * 128, num_idxs_reg=cs * 128,
                                elem_size=128)
                            for b in range(cs):
                                parts.append((gt, b, w * nbx + s0 + b, csb, ssb))
                    acc = psW.tile([128, cfg["win"]], f32, tag="acc")
                    for k, (gt, b, B, csb, ssb) in enumerate(parts):
                        oh = poh.tile([128, cfg["win"]], f32r, tag="oh")
                        nc.vector.tensor_scalar(
                            oh[:], iota_sb[:], csb[:, B:B + 1], ssb[:, B:B + 1],
                            AluOpType.is_equal, AluOpType.mult)
                        nc.tensor.matmul(acc[:], gt[:, b, :], oh[:],
                                         start=(k == 0), stop=(k == len(parts) - 1))
                    nc.scalar.activation(xTnext[:, w * cfg["win"]:(w + 1) * cfg["win"]],
                                         acc[:], AF.Relu, bias=bias_sb[:, 0:1])

            if "G1" in PHASES:
                agg_layer(T1, x2T, b1_sb)

            # ---- layer-2 table: local shard + AllGather ----
            for t in range(nsh_t if "S" in PHASES else 0):
                ps = psA.tile([128, 128], f32, tag="psA")
                nc.tensor.matmul(ps[:], x2T[:, t * 128:(t + 1) * 128], W2_sb[:],
                                 start=True, stop=True)
                h = pha.tile([128, 128], f32r, tag="ha")
                nc.vector.tensor_scalar(h[:], ps[:], disloc_sb[:, t:t + 1], None,
                                        AluOpType.mult)
                nc.sync.dma_start(bounce[t * 128:(t + 1) * 128, :], h[:])
            if NO_AG or "S" not in PHASES:
                for t in range(nsh_t if "S" in PHASES else 0):
                    h = pha.tile([128, 128], f32r, tag="ha")
                    nc.sync.dma_start(h[:], bounce[t * 128:(t + 1) * 128, :])
                    nc.sync.dma_start(T2[t * 128:(t + 1) * 128, :], h[:])
            else:
                nc.gpsimd.collective_compute(
                    "AllGather", mybir.AluOpType.bypass,
                    replica_groups=[list(range(NCORES))],
                    ins=[bounce[:]], outs=[T2[:]])

            if "G2" in PHASES:
                agg_layer(T2, x3T, b2_sb)

            # ---- head ----
            for t in range(nsh_t):
                ps = psH.tile([128, 3], f32, tag="psH")
                nc.tensor.matmul(ps[:], x3T[:, t * 128:(t + 1) * 128], Wh_sb[:],
                                 start=True, stop=True)
                nc.vector.tensor_tensor(out_sb[:, t * 3:(t + 1) * 3], ps[:], bh_sb[:],
                                        AluOpType.add)
            nc.sync.dma_start(out[:], out_sb[:])

    nc.compile()
    return nc


def kernel(x, edge_index, W1, b1, W2, b2, Wh, bh, cfg=_CFG_FULL, _trace=False):
    from concourse.bass_utils import run_bass_kernel_spmd

    x = np.asarray(x, dtype=np.float32)
    W1 = np.asarray(W1, np.float32); b1 = np.asarray(b1, np.float32)
    W2 = np.asarray(W2, np.float32); b2 = np.asarray(b2, np.float32)
    Wh = np.asarray(Wh, np.float32); bh = np.asarray(bh, np.float32)
    n, nsh, win = cfg["n"], cfg["nsh"], cfg["win"]
    nloc, npad, nwin, ntile = _derive(cfg)

    cores, degt_t, nbl, nbh, t_of_r = prep(edge_index, cfg)
    nc = build_nc(nbl, nbh, cfg)

    xTp = np.zeros((128, npad), np.float32)
    xTp[:, t_of_r] = x.T  # table-order, feature-major
    iota_np = np.tile(np.arange(win, dtype=np.float32), (128, 1))
    shared = dict(
        xT=xTp, W1=W1, W2=W2, Wh=Wh,
        b1=b1.reshape(128, 1), b2=b2.reshape(128, 1),
        bh=np.tile(bh.reshape(1, 3), (128, 1)).copy(),
        degt=degt_t, iota=iota_np,
    )
    in_maps = [dict(shared, **cores[c]) for c in range(NCORES)]
    res = run_bass_kernel_spmd(nc, in_maps, core_ids=list(range(NCORES)),
                               trace=_trace)

    outs = []
    for c in range(NCORES):
        o = res.results[c]["out"].reshape(128, nloc // 128, 3)
        outs.append(o.transpose(1, 0, 2).reshape(nloc, 3)[:nsh])
    full = np.concatenate(outs, axis=0)[:n]
    if _trace:
        kernel.last_exec_ns = res.exec_time_ns
        kernel.last_trace = (res.instructions_and_trace or (None, None))[1]
    return full



# revision 7
# speedup vs baseline: 1.2337x; 1.2337x over previous
"""GCN (2x GCNConv + linear head) on 8 TRN2 NeuronCores — bf16 pipeline.

Strategy (graph-parallel by target node):
- Nodes sharded across 8 cores (6250 real + pad = 6400 rows/core,
  table_row = core*6400 + local).
- Layer tables H = dis_src * (x @ W) live in DRAM as bf16 rows (256B),
  built as local shards then AllGathered (both layers).
- Edges grouped per (core, 256-target window, lo/hi source range); the
  lo/hi split works around dma_gather's int16 index range.  Each
  128-edge block becomes one bf16 matmul: PSUM[feat, tgt] +=
  gathered[edge, feat].T @ onehot[edge, tgt], onehot built by a single
  DVE is_equal (bf16 4x mode).  Per-window drain applies the target-deg
  scale (tensor_tensor with a broadcast rsqrt(deg) tile), then ReLU+bias
  on the Scalar engine.  Self-loops are ordinary edges.
- Window block counts vary per window (max over cores keeps the SPMD
  program identical); gather calls pack GCH blocks flat across windows.
- Host prep is integer-only; rsqrt(deg) is computed on device.
"""

import numpy as np
from ml_dtypes import bfloat16

N_REAL = 50000
E_REAL = 800000
D = 128
NCORES = 8
GCH = 8  # blocks per dma_gather call (1024 idxs: swdge firmware limit)
PHASES = {"A", "G1", "S", "G2", "H"}

_CFG_FULL = dict(n=N_REAL, nsh=6250, win=256, split=32768)


def _derive(cfg):
    nsh = cfg["nsh"]
    nloc = ((nsh + 127) // 128) * 128
    win = cfg["win"]
    nloc = ((nloc + win - 1) // win) * win
    npad = NCORES * nloc
    return nloc, npad, nloc // win, npad // 128


def _calls(nb):
    """Pack nb blocks into dma_gather calls of <= GCH blocks."""
    out = []
    b0 = 0
    while b0 < nb:
        out.append((b0, min(GCH, nb - b0)))
        b0 += GCH
    return out


def prep(edge_index, cfg=_CFG_FULL):
    """Host-side (integer-only) graph preprocessing -> per-core arrays."""
    n, nsh, win, split = cfg["n"], cfg["nsh"], cfg["win"], cfg["split"]
    nloc, npad, nwin, _ = _derive(cfg)
    row = np.asarray(edge_index[0]).astype(np.int64)
    col = np.asarray(edge_index[1]).astype(np.int64)

    deg = np.bincount(col, minlength=n).astype(np.float32) + 1.0  # + self loop
    deg_t = np.ones(npad, np.float32)
    rr = np.arange(n, dtype=np.int64)
    t_of_r = (rr // nsh) * nloc + (rr % nsh)
    deg_t[t_of_r] = deg

    trow = (row // nsh) * nloc + (row % nsh)
    tcol = (col // nsh) * nloc + (col % nsh)
    core_of = col // nsh

    # per-core/per-class edge lists sorted by window
    percls = []  # [core][cls] = (src_rows, col_in_window, window_bounds)
    for c in range(NCORES):
        m = core_of == c
        er = np.concatenate([trow[m], c * nloc + np.arange(nsh, dtype=np.int64)])
        ecl = np.concatenate([tcol[m] - c * nloc, np.arange(nsh, dtype=np.int64)])
        w = ecl // win
        is_hi = er >= split
        cls_list = []
        for cls in (0, 1):
            mm = is_hi == bool(cls)
            erc, eclc, wc = er[mm], ecl[mm], w[mm]
            order = np.argsort(wc, kind="stable")
            erc, eclc, wc = erc[order], eclc[order], wc[order]
            bounds = np.searchsorted(wc, np.arange(nwin + 1))
            cls_list.append((erc, eclc, bounds))
        percls.append(cls_list)

    # per-(class, window) block count = max over cores (keeps SPMD structure)
    nbw = np.zeros((2, nwin), np.int64)
    for cls in (0, 1):
        for w in range(nwin):
            mx = max(percls[c][cls][2][w + 1] - percls[c][cls][2][w]
                     for c in range(NCORES))
            nbw[cls, w] = (mx + 127) // 128
    assert (nbw.sum(axis=0) > 0).all()

    starts = [np.concatenate([[0], np.cumsum(nbw[cls])]) for cls in (0, 1)]
    NB = [int(starts[cls][-1]) for cls in (0, 1)]

    cores = []
    for c in range(NCORES):
        arrs = {}
        for cls, sfx in ((0, "lo"), (1, "hi")):
            erc, eclc, bounds = percls[c][cls]
            nb = NB[cls]
            src = np.zeros((nb, 128), np.int64)
            cw = np.full((nb, 128), -1.0, np.float32)
            for w in range(nwin):
                a, b = bounds[w], bounds[w + 1]
                k = b - a
                base = starts[cls][w] * 128
                flat_s = src.reshape(-1)
                flat_c = cw.reshape(-1)
                flat_s[base:base + k] = erc[a:b] - (split if cls else 0)
                flat_c[base:base + k] = (eclc[a:b] % win).astype(np.float32)
            # gather idx layout per call: idx e -> part e%16 (replicated x8),
            # col e//16
            gparts = []
            for b0, cs in _calls(nb):
                s = src[b0:b0 + cs].reshape(-1)
                g16 = s.reshape(-1, 16).T  # [16, cs*8]
                gparts.append(np.tile(g16, (8, 1)))
            arrs[f"gidx_{sfx}"] = np.concatenate(gparts, axis=1).astype(np.int16)
            arrs[f"cw_{sfx}"] = cw.T.copy()  # f32: is_equal needs f32 scalar
        # deg of this core's own table rows, [128, nloc/128] tiled
        arrs["degloc"] = deg_t[c * nloc:(c + 1) * nloc].reshape(-1, 128).T.copy()
        # deg of this core's targets broadcast across partitions
        arrs["degb"] = np.tile(deg_t[c * nloc:(c + 1) * nloc], (128, 1)).copy()
        cores.append(arrs)

    return cores, nbw, t_of_r


def build_nc(nbw, cfg=_CFG_FULL):
    import concourse.bacc as bacc
    import concourse.tile as tile
    import concourse.mybir as mybir
    from concourse.alu_op_type import AluOpType

    nloc, npad, nwin, ntile = _derive(cfg)
    win, split = cfg["win"], cfg["split"]
    f32 = mybir.dt.float32
    bf16 = mybir.dt.bfloat16
    i16 = mybir.dt.int16
    AF = mybir.ActivationFunctionType
    nsh_t = nloc // 128

    starts = [np.concatenate([[0], np.cumsum(nbw[cls])]) for cls in (0, 1)]
    NB = [int(starts[cls][-1]) for cls in (0, 1)]
    calls = [_calls(NB[cls]) for cls in (0, 1)]
    call_of_block = []
    for cls in (0, 1):
        m = {}
        for j, (b0, cs) in enumerate(calls[cls]):
            for s in range(cs):
                m[b0 + s] = (j, s)
        call_of_block.append(m)

    nc = bacc.Bacc("TRN2", target_bir_lowering=False, debug=False,
                   num_devices=NCORES)
    inp = {}

    def I(name, shape, dt=f32):
        inp[name] = nc.dram_tensor(name, list(shape), dt, kind="ExternalInput").ap()
        return inp[name]

    xloc = I("xloc", [128, nloc], bf16)
    W1 = I("W1", [128, 128], bf16); W2 = I("W2", [128, 128], bf16)
    Wh = I("Wh", [128, 3], bf16)
    b1 = I("b1", [128, 1]); b2 = I("b2", [128, 1]); bh = I("bh", [128, 3])
    degloc = I("degloc", [128, nsh_t]); degb = I("degb", [128, nloc])
    iota = I("iota", [128, win], bf16)
    g_lo = I("gidx_lo", [128, NB[0] * 8], i16)
    g_hi = I("gidx_hi", [128, NB[1] * 8], i16)
    c_lo = I("cw_lo", [128, NB[0]]); c_hi = I("cw_hi", [128, NB[1]])
    out = nc.dram_tensor("out", [128, nsh_t * 3], f32, kind="ExternalOutput").ap()

    bounce1 = nc.dram_tensor("bounce1", [nloc, 128], bf16, kind="Internal").ap()
    bounce2 = nc.dram_tensor("bounce2", [nloc, 128], bf16, kind="Internal").ap()
    T1 = nc.dram_tensor("T1", [npad, 128], bf16, kind="Internal",
                        addr_space="Shared").ap()
    T2 = nc.dram_tensor("T2", [npad, 128], bf16, kind="Internal",
                        addr_space="Shared").ap()

    with tile.TileContext(nc) as tc:
        with (
            tc.tile_pool(name="const", bufs=1) as pc,
            tc.tile_pool(name="ha", bufs=3) as pha,
            tc.tile_pool(name="glo", bufs=3) as pglo,
            tc.tile_pool(name="ghi", bufs=3) as pghi,
            tc.tile_pool(name="oh", bufs=6) as poh,
            tc.tile_pool(name="tmp", bufs=2) as ptmp,
            tc.tile_pool(name="act", bufs=1) as pact,
            tc.tile_pool(name="psA", bufs=2, space="PSUM") as psA,
            tc.tile_pool(name="psW", bufs=2, space="PSUM") as psW,
            tc.tile_pool(name="psH", bufs=2, space="PSUM") as psH,
        ):
            def load(ap, shape, tag, dt=f32):
                t = pc.tile(shape, dt, tag=tag)
                nc.sync.dma_start(t[:], ap[:])
                return t

            iota_sb = load(iota, [128, win], "iota", bf16)
            W1_sb = load(W1, [128, 128], "W1", bf16)
            W2_sb = load(W2, [128, 128], "W2", bf16)
            Wh_sb = load(Wh, [128, 3], "Wh", bf16)
            b1_sb = load(b1, [128, 1], "b1"); b2_sb = load(b2, [128, 1], "b2")
            bh_sb = load(bh, [128, 3], "bh")
            glo_sb = load(g_lo, [128, NB[0] * 8], "glosb", i16)
            ghi_sb = load(g_hi, [128, NB[1] * 8], "ghisb", i16)
            cw_sb = [load(c_lo, [128, NB[0]], "closb"),
                     load(c_hi, [128, NB[1]], "chisb")]
            xloc_sb = load(xloc, [128, nloc], "xloc", bf16)

            def rsqrt_of(ap, cols, tag):
                dsb = load(ap, [128, cols], tag + "_d")
                nc.vector.reciprocal(dsb[:], dsb[:])
                nc.scalar.activation(dsb[:], dsb[:], AF.Sqrt)
                return dsb

            disloc_sb = rsqrt_of(degloc, nsh_t, "disl")
            disb_sb = rsqrt_of(degb, nloc, "disb")

            # persistent activations (feature-major)
            x2T = pact.tile([128, nloc], bf16, tag="x2T")
            x3T = pact.tile([128, nloc], bf16, tag="x3T")
            out_sb = pact.tile([128, nsh_t * 3], f32, tag="osb")

            # ---- local table shard: rows t*128..t*128+127 of this core ----
            def table_shard(src_sb, W_sb, bounce):
                for t in range(nsh_t):
                    ps = psA.tile([128, 128], f32, tag="psA")
                    nc.tensor.matmul(ps[:], src_sb[:, t * 128:(t + 1) * 128],
                                     W_sb[:], start=True, stop=True)
                    h = pha.tile([128, 128], bf16, tag="ha")
                    nc.vector.tensor_scalar(h[:], ps[:], disloc_sb[:, t:t + 1],
                                            None, AluOpType.mult)
                    nc.sync.dma_start(bounce[t * 128:(t + 1) * 128, :], h[:])

            def allgather(bounce, T):
                nc.gpsimd.collective_compute(
                    "AllGather", mybir.AluOpType.bypass,
                    replica_groups=[list(range(NCORES))],
                    ins=[bounce[:]], outs=[T[:]])

            if "A" in PHASES:
                table_shard(xloc_sb, W1_sb, bounce1)
                allgather(bounce1, T1)

            # ---- one GCN aggregation layer ----
            def agg_layer(T, xTnext, bias_sb):
                emitted = [{}, {}]  # cls -> call j -> gather tile

                def ensure_call(cls, j):
                    if j in emitted[cls]:
                        return emitted[cls][j]
                    b0, cs = calls[cls][j]
                    pl = pglo if cls == 0 else pghi
                    gsb = glo_sb if cls == 0 else ghi_sb
                    lim = (0, split) if cls == 0 else (split, npad)
                    gt = pl.tile([128, cs, 128], bf16, tag=f"g{cls}_{j % 3}")
                    nc.gpsimd.dma_gather(
                        gt[:], T[lim[0]:lim[1], :], gsb[:, b0 * 8:(b0 + cs) * 8],
                        num_idxs=cs * 128, num_idxs_reg=cs * 128,
                        elem_size=128)
                    emitted[cls][j] = gt
                    return gt

                for w in range(nwin):
                    parts = []
                    for cls in (0, 1):
                        for b in range(int(starts[cls][w]), int(starts[cls][w + 1])):
                            j, slot = call_of_block[cls][b]
                            gt = ensure_call(cls, j)
                            parts.append((gt, slot, cls, b))
                    acc = psW.tile([128, win], f32, tag="acc")
                    for k, (gt, slot, cls, b) in enumerate(parts):
                        oh = poh.tile([128, win], bf16, tag="oh")
                        nc.vector.tensor_scalar(oh[:], iota_sb[:],
                                                cw_sb[cls][:, b:b + 1], None,
                                                AluOpType.is_equal)
                        nc.tensor.matmul(acc[:], gt[:, slot, :], oh[:],
                                         start=(k == 0), stop=(k == len(parts) - 1))
                    tmp = ptmp.tile([128, win], bf16, tag="tmp")
                    nc.vector.tensor_tensor(tmp[:], acc[:],
                                            disb_sb[:, w * win:(w + 1) * win],
                                            AluOpType.mult)
                    nc.scalar.activation(xTnext[:, w * win:(w + 1) * win],
                                         tmp[:], AF.Relu, bias=bias_sb[:, 0:1])

            if "G1" in PHASES:
                agg_layer(T1, x2T, b1_sb)
            if "S" in PHASES:
                table_shard(x2T, W2_sb, bounce2)
                allgather(bounce2, T2)
            if "G2" in PHASES:
                agg_layer(T2, x3T, b2_sb)

            # ---- head ----
            for t in range(nsh_t if "H" in PHASES else 0):
                ps = psH.tile([128, 3], f32, tag="psH")
                nc.tensor.matmul(ps[:], x3T[:, t * 128:(t + 1) * 128], Wh_sb[:],
                                 start=True, stop=True)
                nc.vector.tensor_tensor(out_sb[:, t * 3:(t + 1) * 3], ps[:],
                                        bh_sb[:], AluOpType.add)
            nc.sync.dma_start(out[:], out_sb[:])

    nc.compile()
    return nc


def kernel(x, edge_index, W1, b1, W2, b2, Wh, bh, cfg=_CFG_FULL, _trace=False):
    from concourse.bass_utils import run_bass_kernel_spmd

    x = np.asarray(x, dtype=np.float32)
    W1 = np.asarray(W1, np.float32); b1 = np.asarray(b1, np.float32)
    W2 = np.asarray(W2, np.float32); b2 = np.asarray(b2, np.float32)
    Wh = np.asarray(Wh, np.float32); bh = np.asarray(bh, np.float32)
    n, nsh, win = cfg["n"], cfg["nsh"], cfg["win"]
    nloc, npad, nwin, ntile = _derive(cfg)

    cores, nbw, t_of_r = prep(edge_index, cfg)
    nc = build_nc(nbw, cfg)

    xTp = np.zeros((128, npad), np.float32)
    xTp[:, t_of_r] = x.T  # table-order, feature-major
    iota_np = np.tile(np.arange(win, dtype=np.float32), (128, 1))
    Wh_p = np.zeros((128, 3), np.float32); Wh_p[:, :] = Wh
    shared = dict(
        W1=W1.astype(bfloat16), W2=W2.astype(bfloat16),
        Wh=Wh.astype(bfloat16),
        b1=b1.reshape(128, 1), b2=b2.reshape(128, 1),
        bh=np.tile(bh.reshape(1, 3), (128, 1)).copy(),
        iota=iota_np.astype(bfloat16),
    )
    in_maps = []
    for c in range(NCORES):
        m = dict(shared, **cores[c])
        m["xloc"] = xTp[:, c * nloc:(c + 1) * nloc].astype(bfloat16)
        in_maps.append(m)
    res = run_bass_kernel_spmd(nc, in_maps, core_ids=list(range(NCORES)),
                               trace=_trace)

    outs = []
    for c in range(NCORES):
        o = res.results[c]["out"].reshape(128, nloc // 128, 3)
        outs.append(o.transpose(1, 0, 2).reshape(nloc, 3)[:nsh])
    full = np.concatenate(outs, axis=0)[:n]
    if _trace:
        kernel.last_exec_ns = res.exec_time_ns
        kernel.last_trace = (res.instructions_and_trace or (None, None))[1]
    return full
